# revision 13
# baseline (speedup 1.0000x reference)
"""BitLinear fake-quant GEMM on 8 TRN2 NeuronCores.

Reference math:
  abs_mean  = mean(|W|);  thr = 0.7*abs_mean
  Wq        = sign(W) * (|W| >= thr)            (ternary)
  scale_w   = abs_mean / (mean(Wq != 0) + 1e-8)
  sx        = 127 / max(|X|)
  Xq        = round(X * sx)                      (integer valued, |.| <= 127)
  out       = (Xq @ Wq^T) * scale_w / sx

Sharding: data-parallel over tokens (8192/8 = 1024 columns of X^T per core);
W is replicated.  The host hands each core PRE-TRANSPOSED operands (x.T shard
and w.T) so both matmul operands already have the contraction dim
(in_features) on partitions; the device performs zero transposes.

GEMM runs in fp8e4m3 DoubleRow mode (2 k-tiles per instruction, 0.5
cycles/row) and stays EXACT via a hi/lo split of the integer activations:
  Xq = a16 + b,  a16 = 16*round(Xq/16) in {-128..128 step 16},  b in [-8,8]
Both parts and the ternary Wq in {-1,0,1} are exactly representable in
fp8e4m3, and fp32 PSUM accumulation of 8192 products of magnitude <= 128
stays below 2^24.  Each PSUM tile accumulates 32 DoubleRow matmuls: 16
k-pairs of a16 + 16 k-pairs of b.

Schedule:
  1. wT stats slice reads first (23us), x shard next (till ~67us); the two
     scalar AllGathers move their operands over the SEQUENCER REGISTER path
     (value_load/store, int32-bitcast) instead of DMA - tiny transfers would
     otherwise queue behind multi-MB bulk transfers on the shared DMA
     engines and delay thr/sx by 20-40us.
  2. DVE: wsl+x reduces (DMA-paced) -> panel 0 quantizes fully in the
     window before/around sx -> x quantizes in group order [5,6,7,0..4]
     (those three groups are still SBUF-resident from the stats pass; the
     rest re-read, pre-staged by the third stgx buffer) -> panels 1..7.
  3. PE consumes k-pairs in the matching order [10..15, 0..9]; quarters
     quantize in order [2,3,0,1] to match.  Steady state: panel p on PE
     (27.3us) while panel p+1 quantizes on DVE (26.1us) - DVE-bound.

The hi/lo parts use the fp32 round-to-nearest-even MAGIC-add trick:
v2 = x*sx + MAGIC encodes MAGIC + Xq (in-place, DVE); u1 = v2/16 +
(15/16)*MAGIC encodes MAGIC + round(Xq/16) (scalar engine); a16 = 16*u1 -
16*MAGIC cast to fp8 (scalar engine); b = (v2 - MAGIC) - a16 (DVE STT).

The nonzero count of Wq falls out of the quantization passes for free via
DVE accum_out side-sums.  The final scalar rescale by scale_w/sx happens on
the host during the unshard; the device stores the output in bf16 (0.2%
relative, well under tolerance) to halve output DMA.

The per-core output is written tile-chunked ([panel][tblock][128][512]); the
host permutes it back during the gather.
"""

from contextlib import ExitStack

import numpy as np

import concourse.bass as bass
import concourse.bass_isa as bass_isa
import concourse.tile as tile
from concourse import bacc, mybir
from concourse.bass import ts as _ts
from concourse.bass_utils import run_bass_kernel_spmd

P = 128
T, I, O = 8192, 4096, 4096  # tokens, in_features, out_features
NC = 8
TSH = T // NC  # 1024 token columns per core
ISL = I // NC  # 512 wT rows per core for stats
NMM = 512  # matmul moving free dim (one fp32 PSUM bank)
MAGIC = 12582912.0  # 1.5 * 2**23: fp32 round-to-nearest-even bias trick

F32 = mybir.dt.float32
BF16 = mybir.dt.bfloat16
FP8 = mybir.dt.float8e4
I32 = mybir.dt.int32
ALU = mybir.AluOpType
AXX = mybir.AxisListType
DR = mybir.MatmulPerfMode.DoubleRow

# x groups quantize in this order (5,6,7 stay resident from the stats pass);
# the PE k-pair order matches it so the GEMM ramp consumes groups as they
# are produced.  Group g covers k-pairs (2g, 2g+1); k-pair kp lives in
# quarter kp//4, so quarters quantize in order [2,3,0,1].
XG_ORDER = [5, 6, 7, 0, 1, 2, 3, 4]
KP_ORDER = [kp for g in XG_ORDER for kp in (2 * g, 2 * g + 1)]
Q_ORDER = [2, 3, 0, 1]


def _bitlinear(tc, out, sout, xT, wT, wsl):
    nc = tc.nc

    with ExitStack() as ctx:
        const = ctx.enter_context(tc.tile_pool(name="const", bufs=1))
        statp = ctx.enter_context(tc.tile_pool(name="statp", bufs=1))
        dram = ctx.enter_context(tc.tile_pool(name="dram", bufs=1, space="DRAM"))
        stgx = ctx.enter_context(tc.tile_pool(name="stgx", bufs=3))   # f32 [128,4,1024]
        stgw = ctx.enter_context(tc.tile_pool(name="stgw", bufs=3))   # f32 [128,8,512]
        b2p = ctx.enter_context(tc.tile_pool(name="b2p", bufs=1))     # fp8 [128,8,512]
        u1p = ctx.enter_context(tc.tile_pool(name="u1p", bufs=1))     # f32 [128,1,1024]
        ap8 = ctx.enter_context(tc.tile_pool(name="ap8", bufs=1))     # fp8 hi groups
        bp8 = ctx.enter_context(tc.tile_pool(name="bp8", bufs=1))     # fp8 lo groups
        wqTp = ctx.enter_context(tc.tile_pool(name="wqTp", bufs=2))   # fp8 quarters
        psum = ctx.enter_context(tc.tile_pool(name="psum", bufs=1, space="PSUM"))
        osb = ctx.enter_context(tc.tile_pool(name="osb", bufs=4))     # bf16 [128,512]

        # constants first: Pool SEQ must not be frozen behind collective waits
        b15mag = const.tile([P, 1], F32)
        nc.gpsimd.memset(b15mag[:], MAGIC * 15.0 / 16.0)
        nmag16 = const.tile([P, 1], F32)
        nc.gpsimd.memset(nmag16[:], -16.0 * MAGIC)

        # ---- Phase 1a: x shard streams in first -> earliest possible sx ----
        xmax_part = statp.tile([P, 8], F32)
        stat_tiles = {}
        for g in range(8):
            xt = stgx.tile([P, 4, TSH], F32, tag="xstage")
            src = xT[g * 512 : (g + 1) * 512, :].rearrange("(c p) t -> p c t", p=P)
            nc.sync.dma_start(xt[:], src)
            nc.vector.tensor_reduce(
                xmax_part[:, g : g + 1], xt[:], axis=AXX.XY, op=ALU.max,
                apply_absolute_value=True,
            )
            stat_tiles[g] = xt
        xmax_c = statp.tile([P, 1], F32)
        nc.vector.tensor_reduce(xmax_c[:], xmax_part[:], axis=AXX.X, op=ALU.max)
        xmax_a = statp.tile([P, 1], F32)
        nc.gpsimd.partition_all_reduce(
            xmax_a[:], xmax_c[:], channels=P, reduce_op=bass_isa.ReduceOp.max
        )
        xcin = dram.tile([1, 1], F32)
        xcout = dram.tile([1, NC], F32)
        nc.gpsimd.dma_start(xcin[:], xmax_a[0:1, 0:1])
        nc.gpsimd.collective_compute(
            "AllGather", ALU.bypass, replica_groups=[list(range(NC))],
            ins=[xcin.opt()], outs=[xcout.opt()],
        )
        xgg = statp.tile([1, NC], F32)
        nc.gpsimd.dma_start(xgg[:], xcout[:])
        gmax = statp.tile([1, 1], F32)
        nc.vector.tensor_reduce(gmax[:], xgg[:], axis=AXX.X, op=ALU.max)
        gmax_c = statp.tile([1, 1], F32)
        nc.vector.tensor_scalar(gmax_c[:], gmax[:], 1e-12, None, op0=ALU.max)
        rec1 = statp.tile([1, 1], F32)
        nc.vector.reciprocal(rec1[:], gmax_c[:])
        sx1 = statp.tile([1, 1], F32)
        nc.vector.tensor_scalar(sx1[:], rec1[:], 127.0, None, op0=ALU.mult)
        sx128 = const.tile([P, 1], F32)
        nc.gpsimd.partition_broadcast(sx128[:], sx1[:])

        # ---- Phase 1b: W stats slice (thr is only needed when panel 0
        # quantizes, ~50us after sx; its whole chain rides in that slack) ----
        wsum_part = statp.tile([P, 4], F32)
        for c in range(4):
            wt = stgw.tile([P, 8, NMM], F32, tag="wstage")
            wsrc = wsl[_ts(c, P), :].rearrange("p (c j) -> p c j", c=8)
            nc.sync.dma_start(wt[:, 0:4, :], wsrc[:, 0:4, :])
            nc.sync.dma_start(wt[:, 4:8, :], wsrc[:, 4:8, :])
            nc.vector.tensor_reduce(
                wsum_part[:, c : c + 1], wt[:], axis=AXX.XY, op=ALU.add,
                apply_absolute_value=True,
            )
        wsum_c = statp.tile([P, 1], F32)
        nc.vector.tensor_reduce(wsum_c[:], wsum_part[:], axis=AXX.X, op=ALU.add)
        wsum_a = statp.tile([P, 1], F32)
        nc.gpsimd.partition_all_reduce(
            wsum_a[:], wsum_c[:], channels=P, reduce_op=bass_isa.ReduceOp.add
        )
        wcin = dram.tile([1, 1], F32)
        wcout = dram.tile([1, NC], F32)
        nc.gpsimd.dma_start(wcin[:], wsum_a[0:1, 0:1])
        nc.gpsimd.collective_compute(
            "AllGather", ALU.bypass, replica_groups=[list(range(NC))],
            ins=[wcin.opt()], outs=[wcout.opt()],
        )
        wgg = statp.tile([1, NC], F32)
        nc.gpsimd.dma_start(wgg[:], wcout[:])
        gsum = statp.tile([1, 1], F32)
        nc.vector.tensor_reduce(gsum[:], wgg[:], axis=AXX.X, op=ALU.add)
        thr1 = statp.tile([1, 1], F32)
        nc.vector.tensor_scalar(thr1[:], gsum[:], 0.7 / float(O * I), None, op0=ALU.mult)
        nthr1 = statp.tile([1, 1], F32)
        nc.vector.tensor_scalar(nthr1[:], thr1[:], -1.0, None, op0=ALU.mult)
        thr128 = const.tile([P, 1], F32)
        nc.gpsimd.partition_broadcast(thr128[:], thr1[:])
        nthr128 = const.tile([P, 1], F32)
        nc.gpsimd.partition_broadcast(nthr128[:], nthr1[:])

        # ---- W panel machinery ----
        qaccs = statp.tile([P, 32], F32)  # sum(Wq) per quarter  ( #pos - #neg )
        naccs = statp.tile([P, 32], F32)  # sum(b2) per quarter  ( #neg )

        def quant_panel(op_, queue="scalar"):
            quarters = [None] * 4
            for q in Q_ORDER:
                col = op_ * 4 + q
                wt = stgw.tile([P, 8, NMM], F32, tag="wstage")
                src = wT[
                    q * 1024 : (q + 1) * 1024, _ts(op_, NMM)
                ].rearrange("(c p) j -> p c j", p=P)
                getattr(nc, queue).dma_start(wt[:], src)
                b2 = b2p.tile([P, 8, NMM], FP8)
                # op1 doubles as the accum_out reduce op (walrus requires it)
                nc.vector.tensor_scalar(
                    b2[:], wt[:], nthr128[:], None, op0=ALU.is_le, op1=ALU.add,
                    accum_out=naccs[:, col : col + 1],
                )
                wq = wqTp.tile([P, 8, NMM], FP8, tag=f"wq{q}")
                nc.vector.scalar_tensor_tensor(
                    wq[:], wt[:], thr128[:], b2[:],
                    op0=ALU.is_ge, op1=ALU.subtract,
                    accum_out=qaccs[:, col : col + 1],
                )
                quarters[q] = wq
            return quarters

        def rhs_pair(quarters, kp):
            q, ci = kp // 4, (kp % 4) * 2
            return quarters[q][:, ci : ci + 2, :]

        # ---- Phase 2: Xq^T hi/lo fp8 (order matches KP_ORDER) ----
        a_groups = [None] * 8
        b_groups = [None] * 8
        for g in XG_ORDER:
            if g >= 5:
                xt = stat_tiles[g]
            else:
                xt = stgx.tile([P, 4, TSH], F32, tag="xstage")
                src = xT[g * 512 : (g + 1) * 512, :].rearrange(
                    "(c p) t -> p c t", p=P
                )
                nc.sync.dma_start(xt[:], src)
            # v2 = x*sx + MAGIC in place: encodes MAGIC + Xq
            nc.vector.tensor_scalar(
                xt[:], xt[:], sx128[:], MAGIC, op0=ALU.mult, op1=ALU.add
            )
            ag = ap8.tile([P, 4, TSH], FP8, tag=f"a{g}", name=f"a{g}")
            for h in range(4):  # quarter-group u1 scratch trims SBUF pressure
                u1 = u1p.tile([P, 1, TSH], F32, tag="u1")
                # u1 = v2/16 + (15/16)*MAGIC: encodes MAGIC + round(Xq/16)
                nc.scalar.activation(
                    u1[:], xt[:, h : h + 1, :],
                    mybir.ActivationFunctionType.Identity,
                    bias=b15mag[:], scale=1.0 / 16.0,
                )
                # a16 = 16*u1 - 16*MAGIC -> fp8 (multiples of 16 in [-128,128])
                nc.scalar.activation(
                    ag[:, h : h + 1, :], u1[:],
                    mybir.ActivationFunctionType.Identity,
                    bias=nmag16[:], scale=16.0,
                )
            # b = (v2 - MAGIC) - a16 -> fp8 (integers in [-8,8])
            bg = bp8.tile([P, 4, TSH], FP8, tag=f"b{g}", name=f"b{g}")
            nc.vector.scalar_tensor_tensor(
                bg[:], xt[:], -MAGIC, ag[:], op0=ALU.add, op1=ALU.subtract
            )
            a_groups[g] = ag
            b_groups[g] = bg

        def lhsT_pair(half, kp, tb):
            g, c = kp // 2, (kp % 2) * 2
            src = a_groups[g] if half == 0 else b_groups[g]
            return src[:, c : c + 2, tb * P : (tb + 1) * P]

        # ---- Phase 3: panel 0 reads ride the sync queue after the x
        # re-reads; later panels are emitted interleaved with the GEMM ----
        panel_q = {0: quant_panel(0, queue="sync")}
        panel_q[1] = quant_panel(1)

        # ---- Phase 4: DoubleRow GEMM ----
        def run_tile(ps, quarters, tb):
            for i, kp in enumerate(KP_ORDER):
                for half in (0, 1):
                    nc.tensor.matmul(
                        ps[:],
                        lhsT=lhsT_pair(half, kp, tb),
                        rhs=rhs_pair(quarters, kp),
                        start=(i == 0 and half == 0),
                        stop=(i == 15 and half == 1),
                        perf_mode=DR,
                    )

        def emit_store(op_, tb, ps):
            ot = osb.tile([P, NMM], BF16)
            nc.scalar.copy(ot[:], ps[:])
            # chunked output: (panel, tb) tile as one contiguous run
            nc.sync.dma_start(out[_ts(op_ * 8 + tb, P), :], ot[:])

        # steady state: quantize panel p+1 (DVE/ACT-dma), then run panel p
        # on PE - every queue's emission order matches execution order
        def gemm_panel(op_):
            for tb in range(8):
                ps = psum.tile([P, NMM], F32, tag=f"ps{tb}", name=f"ps_{tb}")
                run_tile(ps, panel_q[op_], tb)
                emit_store(op_, tb, ps)

        # ramp: panel 0 kp-outer across all 8 banks so PE consumption tracks
        # the x-quant production order group by group
        ps_tiles = [
            psum.tile([P, NMM], F32, tag=f"ps{tb}", name=f"ps_{tb}")
            for tb in range(8)
        ]
        for i, kp in enumerate(KP_ORDER):
            for half in (0, 1):
                for tb in range(8):
                    nc.tensor.matmul(
                        ps_tiles[tb][:],
                        lhsT=lhsT_pair(half, kp, tb),
                        rhs=rhs_pair(panel_q[0], kp),
                        start=(i == 0 and half == 0),
                        stop=(i == 15 and half == 1),
                        perf_mode=DR,
                    )
        for tb in range(8):
            emit_store(0, tb, ps_tiles[tb])

        for op_ in range(2, 8):
            panel_q[op_] = quant_panel(op_)
            gemm_panel(op_ - 1)
        gemm_panel(7)

        # ---- finalize nonzero count: nnz = sum(Wq) + 2*sum(b2) ----
        qacc_c = statp.tile([P, 1], F32)
        nc.vector.tensor_reduce(qacc_c[:], qaccs[:], axis=AXX.X, op=ALU.add)
        nacc_c = statp.tile([P, 1], F32)
        nc.vector.tensor_reduce(nacc_c[:], naccs[:], axis=AXX.X, op=ALU.add)
        nnz_c = statp.tile([P, 1], F32)
        nc.vector.scalar_tensor_tensor(
            nnz_c[:], nacc_c[:], 2.0, qacc_c[:], op0=ALU.mult, op1=ALU.add
        )
        nnz_a = statp.tile([P, 1], F32)
        nc.gpsimd.partition_all_reduce(
            nnz_a[:], nnz_c[:], channels=P, reduce_op=bass_isa.ReduceOp.add
        )
        # stats outputs last - they must never contend with bulk DMA
        nc.sync.dma_start(sout[0:1, 0:1], gsum[:])
        nc.sync.dma_start(sout[0:1, 1:2], gmax[:])
        nc.sync.dma_start(sout[0:1, 2:3], sx1[:])
        nc.sync.dma_start(sout[0:1, 3:4], nnz_a[0:1, 0:1])


def _build():
    nc = bacc.Bacc("TRN2", debug=False, enable_asserts=False, num_devices=NC)
    xT_ap = nc.dram_tensor("xT_shard", (I, TSH), F32, kind="ExternalInput").ap()
    wT_ap = nc.dram_tensor("wT_full", (I, O), F32, kind="ExternalInput").ap()
    wsl_ap = nc.dram_tensor("wT_slice", (ISL, O), F32, kind="ExternalInput").ap()
    # chunked layout: row (panel*8 + tb)*128 + r, col c  <->  out[tb*128+r, panel*512+c]
    out_ap = nc.dram_tensor("out_shard", (64 * P, NMM), BF16, kind="ExternalOutput").ap()
    st_ap = nc.dram_tensor("stats_out", (1, 4), F32, kind="ExternalOutput").ap()
    with tile.TileContext(nc) as tc:
        _bitlinear(tc, out_ap, st_ap, xT_ap, wT_ap, wsl_ap)
    nc.compile()
    return nc


_NC_CACHE = None


def _get_nc():
    global _NC_CACHE
    if _NC_CACHE is None:
        _NC_CACHE = _build()
    return _NC_CACHE


def _run(x, weight, **spmd_kwargs):
    x = np.ascontiguousarray(np.asarray(x, dtype=np.float32))
    w = np.asarray(weight, dtype=np.float32)
    assert x.shape == (T, I) and w.shape == (O, I)
    nc = _get_nc()
    wT = np.ascontiguousarray(w.T)  # [I, O]
    in_maps = [
        {
            # per-shard transpose directly (cheaper than x.T then slicing)
            "xT_shard": np.ascontiguousarray(x[k * TSH : (k + 1) * TSH].T),
            "wT_full": wT,
            "wT_slice": wT[k * ISL : (k + 1) * ISL],  # contiguous view
        }
        for k in range(NC)
    ]
    res = run_bass_kernel_spmd(nc, in_maps, core_ids=list(range(NC)), **spmd_kwargs)
    outs = res.results

    st0 = outs[0]["stats_out"][0]
    gsum, sx = float(st0[0]), float(st0[2])
    nnz = float(st0[3])  # every core computed the exact global count

    # replicate the reference's fp32 scalar arithmetic
    f32 = np.float32
    n_el = f32(float(O) * float(I))
    abs_mean = f32(f32(gsum) / n_el)
    non_zero_mean = f32(f32(f32(nnz) / n_el) + f32(1e-8))
    scale_w = f32(abs_mean / non_zero_mean)
    scale = f32(np.float64(scale_w) / np.float64(sx))

    # un-chunk each core's [8 panels][8 tb][128][512] output and stack shards
    out = np.empty((T, O), dtype=np.float32)
    for k in range(NC):
        chunk = outs[k]["out_shard"].astype(np.float32).reshape(8, 8, P, NMM)
        out[k * TSH : (k + 1) * TSH] = (
            chunk.transpose(1, 2, 0, 3).reshape(TSH, O)
        )
    out *= scale
    return out, res


def kernel(x, weight):
    out, _ = _run(x, weight)
    return out
